# revision 7
# baseline (speedup 1.0000x reference)
"""GRU encoder (nn_Encoder_26087631356042) Bass/Trainium2 kernel.

Strategy: data-parallel over batch (B=128 -> 16 per core, 8 cores, no
collectives), everything in feature-major "packed" layout
(gate feature f -> (block m = f//128, partition p = f%128)) so the gate
elementwise ops run with 128 active partitions and tiny free dims.

Per 16-step chunk, the input projection (W_ih @ x + bias) is matmul'd
DIRECTLY INTO PSUM slabs (xr / xz / xn as separate tiles so tile-granular
dependencies never serialize the sigmoid against unrelated matmuls).  The
per-step recurrent matmuls then accumulate W_hh @ h on top of the slab
slice with start=False, which removes the xp+hp vector add from the
serial chain entirely.

The z-gate's weight/bias rows are sign-flipped on the host so one sigmoid
directly yields zc = 1-z; z*h is reconstructed off the critical path as
h - zc*h.  All operands are single bf16 (fp32 PSUM accumulation):
measured end-to-end rel err ~8e-3 vs the fp32 reference.
"""

import numpy as np
import ml_dtypes
from contextlib import ExitStack

import concourse.bass as bass
import concourse.bacc as bacc
import concourse.tile as tile
import concourse.mybir as mybir
from concourse.bass_utils import run_bass_kernel_spmd

F32 = mybir.dt.float32
BF16 = mybir.dt.bfloat16
AF = mybir.ActivationFunctionType

B, T, X, H = 128, 2048, 128, 256
G = 3 * H
NCORES = 8
BL = B // NCORES   # 16 batch rows per core
P = 128
CH = 16            # timesteps per PSUM slab chunk
CHB = CH * BL      # 256 slab columns per chunk

bf16 = ml_dtypes.bfloat16


def _build_program(t_steps: int, reps: int = 1, ninner: int = 8,
                   fold: bool = False):
    """Emit the per-core program (same program on all cores; data differs)."""
    nchunks = t_steps // CH
    assert nchunks % ninner == 0
    nouter = nchunks // ninner
    nc = bacc.Bacc(
        "TRN2", target_bir_lowering=False, debug=False, num_devices=NCORES
    )

    # DRAM I/O
    d_xin = nc.dram_tensor("xin", [P, t_steps * BL], BF16, kind="ExternalInput")
    d_whh = nc.dram_tensor("whh", [P, 2 * G], BF16, kind="ExternalInput")
    d_wih = nc.dram_tensor("wih", [P, G], BF16, kind="ExternalInput")
    d_bmat = nc.dram_tensor("bmat", [P, P], BF16, kind="ExternalInput")
    d_seln = nc.dram_tensor("seln", [P, 2 * BL], BF16, kind="ExternalInput")
    d_selr = nc.dram_tensor("selr", [P, 2 * CHB], BF16, kind="ExternalInput")
    d_selz = nc.dram_tensor("selz", [P, 2 * CHB], BF16, kind="ExternalInput")
    d_selxn = nc.dram_tensor("selxn", [P, 2 * CHB], BF16, kind="ExternalInput")
    d_out = nc.dram_tensor("hout", [P, 2 * BL], F32, kind="ExternalOutput")

    with tile.TileContext(nc) as tc, ExitStack() as ctx:
        cpool = ctx.enter_context(tc.tile_pool(name="const", bufs=1))
        state = ctx.enter_context(tc.tile_pool(name="state", bufs=1))
        xinp = ctx.enter_context(tc.tile_pool(name="xin", bufs=2))
        xnp = ctx.enter_context(tc.tile_pool(name="xn", bufs=2))
        gsb = ctx.enter_context(tc.tile_pool(name="gates", bufs=2))
        pxr = ctx.enter_context(tc.tile_pool(name="pxr", bufs=2, space="PSUM"))
        pxz = ctx.enter_context(tc.tile_pool(name="pxz", bufs=2, space="PSUM"))
        pxn = ctx.enter_context(tc.tile_pool(name="pxn", bufs=2, space="PSUM"))
        phpn = ctx.enter_context(tc.tile_pool(name="phpn", bufs=2, space="PSUM"))

        # Constants -> SBUF
        whh = cpool.tile([P, 2 * G], BF16, tag="whh")
        wih = cpool.tile([P, G], BF16, tag="wih")
        bmat = cpool.tile([P, P], BF16, tag="bmat")
        seln = cpool.tile([P, 2 * BL], BF16, tag="seln")
        selr = cpool.tile([P, 2 * CHB], BF16, tag="selr")
        selz = cpool.tile([P, 2 * CHB], BF16, tag="selz")
        selxn = cpool.tile([P, 2 * CHB], BF16, tag="selxn")
        for dst, src in [(whh, d_whh), (wih, d_wih), (bmat, d_bmat),
                         (seln, d_seln), (selr, d_selr), (selz, d_selz),
                         (selxn, d_selxn)]:
            nc.sync.dma_start(dst[:], src.ap()[:])

        # Hidden state (feature-major packed): [128, 2 k-blocks, 16 batch]
        # bf16 ping-pong; CH is even so every chunk starts/ends on idx 0.
        hT = [state.tile([P, 2, BL], BF16, name=f"hT{j}", tag=f"hT{j}")
              for j in range(2)]
        nc.gpsimd.memset(hT[0][:], 0)

        def emit_chunk(coff):
            """coff: scalar expr for the chunk index."""
            # ---- Phase 1: x-projection + biases into PSUM slabs ----
            xt = xinp.tile([P, CHB], BF16, tag="xt")
            nc.sync.dma_start(xt[:], d_xin.ap()[:, bass.ds(coff * CHB, CHB)])
            xr = pxr.tile([P, 2, CHB], F32, tag="xr")
            xz = pxz.tile([P, 2, CHB], F32, tag="xz")
            xpn = pxn.tile([P, 2, CHB], F32, tag="xpn")
            for j, (t_, mbase, sel) in enumerate(
                ((xr, 0, selr), (xz, 2, selz), (xpn, 4, selxn))
            ):
                # xr/xz stay "open" across the whole chunk (per-step h-matmuls
                # keep accumulating while earlier slices are already being
                # read) -> bypass the sim's zone-granular group check; the
                # per-element pending-zero/has_written semantics are what both
                # the sim's value model and the HW actually use.
                sk = t_ is not xpn
                for m in range(2):
                    nc.tensor.matmul(
                        t_[:, m, :], wih[:, 128 * (mbase + m):128 * (mbase + m + 1)],
                        xt[:], start=(m == 0), stop=False, skip_group_check=sk)
                nc.tensor.matmul(t_.rearrange("p a b -> p (a b)"), bmat[:],
                                 sel[:], start=False, stop=(t_ is xpn),
                                 skip_group_check=sk)
            xn = xnp.tile([P, 2, CHB], BF16, tag="xn")
            nc.vector.tensor_copy(xn[:], xpn[:])

            # ---- Recurrence over CH steps ----
            for s in range(CH):
                cur, nxt = s % 2, (s + 1) % 2
                hpn = phpn.tile([P, 2, BL], F32, tag="hpn")
                nc.tensor.matmul(hpn.rearrange("p a b -> p (a b)"), bmat[:],
                                 seln[:], start=True, stop=False)
                sl = slice(s * BL, (s + 1) * BL)

                def hmm(t_, mbase, last=False):
                    for mi in range(2):
                        m = mbase + mi
                        for k in range(2):
                            stop = last and mi == 1 and k == 1
                            o = (t_[:, mi, sl] if t_ is not hpn
                                 else t_[:, mi, :])
                            nc.tensor.matmul(
                                o, whh[:, G * k + 128 * m:G * k + 128 * (m + 1)],
                                hT[cur][:, k, :], start=False, stop=stop,
                                skip_group_check=t_ is not hpn)

                # order: r-blocks, n-blocks, z-blocks (sigmoid_r overlaps
                # the n matmuls; sigmoid_zc overlaps the tanh path).
                # stop=True on the slab banks' last write (final step) closes
                # the accumulation zone before the buffer is reused.
                hmm(xr, 0, last=(s == CH - 1))
                hmm(hpn, 4, last=True)
                hmm(xz, 2, last=(s == CH - 1))

                rv = gsb.tile([P, 2, BL], BF16, tag="rv")
                zc = gsb.tile([P, 2, BL], BF16, tag="zc")
                t1 = gsb.tile([P, 2, BL], BF16, tag="t1")
                t2 = gsb.tile([P, 2, BL], BF16, tag="t2")
                nn = gsb.tile([P, 2, BL], BF16, tag="nn")
                w1 = gsb.tile([P, 2, BL], BF16, tag="w1")
                zh = gsb.tile([P, 2, BL], BF16, tag="zh")
                t3 = gsb.tile([P, 2, BL], BF16, tag="t3")

                nc.scalar.activation(rv[:], xr[:, :, sl], AF.Sigmoid)
                nc.vector.tensor_mul(t1[:], rv[:], hpn[:])
                nc.vector.tensor_add(t2[:], t1[:], xn[:, :, sl])
                nc.scalar.activation(zc[:], xz[:, :, sl], AF.Sigmoid)
                nc.scalar.activation(nn[:], t2[:], AF.Tanh)
                # off-critical-path: zh = z*h = h - zc*h
                nc.vector.tensor_mul(w1[:], zc[:], hT[cur][:])
                nc.vector.tensor_sub(zh[:], hT[cur][:], w1[:])
                nc.vector.tensor_mul(t3[:], zc[:], nn[:])
                nc.vector.tensor_add(hT[nxt][:], t3[:], zh[:])

        def emit_loop():
            with tc.For_i(
                0, nouter,
                hint_engines=(mybir.EngineType.PE, mybir.EngineType.DVE,
                              mybir.EngineType.Activation),
            ) as oi:
                for inner in range(ninner):
                    emit_chunk(oi * ninner + inner)

        if reps > 1:
            with tc.For_i(0, reps, name="rep"):
                emit_loop()
        else:
            emit_loop()

        hout = cpool.tile([P, 2 * BL], F32, tag="hout")
        nc.vector.tensor_copy(hout[:], hT[0].rearrange("p a b -> p (a b)"))
        nc.sync.dma_start(d_out.ap()[:], hout[:])

    nc.compile()
    return nc


_PROGRAM_CACHE: dict = {}


def _get_program(t_steps: int, reps: int = 1, ninner: int = 8):
    key = (t_steps, reps, ninner)
    if key not in _PROGRAM_CACHE:
        _PROGRAM_CACHE[key] = _build_program(t_steps, reps, ninner)
    return _PROGRAM_CACHE[key]


def _pack_inputs(input, W_ih, W_hh, b_ih, b_hh, t_steps: int = T):
    """Host-side packing. Returns per-core in_maps."""
    input = np.asarray(input, np.float32)
    W_ih = np.asarray(W_ih, np.float32)
    W_hh = np.asarray(W_hh, np.float32)
    b_ih = np.asarray(b_ih, np.float32)
    b_hh = np.asarray(b_hh, np.float32)

    # z-gate sign flip
    Whf = W_hh.copy(); Whf[H:2 * H] *= -1.0
    Wif = W_ih.copy(); Wif[H:2 * H] *= -1.0

    # weights, feature-major packed (shared by all cores)
    whhT = np.ascontiguousarray(Whf.T)               # [H, G]
    whh = whhT.reshape(2, P, G).transpose(1, 0, 2).reshape(P, 2 * G)
    whh = whh.astype(bf16)
    wih = np.ascontiguousarray(Wif.T).astype(bf16)   # [X=128, G]

    # bias matrix rows: 0-3 rz (b_ih+b_hh, z flipped), 4-5 b_hh_n, 6-7 b_ih_n
    brz = (b_ih[:2 * H] + b_hh[:2 * H]).copy(); brz[H:] *= -1.0
    bmat32 = np.zeros((P, P), np.float32)
    bmat32[0:4, :] = brz.reshape(4, P)
    bmat32[4:6, :] = b_hh[2 * H:].reshape(2, P)
    bmat32[6:8, :] = b_ih[2 * H:].reshape(2, P)
    bmat = bmat32.astype(bf16)

    seln = np.zeros((P, 2 * BL), np.float32)
    seln[4, :BL] = 1.0; seln[5, BL:] = 1.0
    slabs = {}
    for name, rows in (("selr", (0, 1)), ("selz", (2, 3)), ("selxn", (6, 7))):
        m_ = np.zeros((P, 2 * CHB), np.float32)
        m_[rows[0], :CHB] = 1.0; m_[rows[1], CHB:] = 1.0
        slabs[name] = m_.astype(bf16)

    shared = dict(whh=whh, wih=wih, bmat=bmat, seln=seln.astype(bf16), **slabs)
    in_maps = []
    for c in range(NCORES):
        xs = input[c * BL:(c + 1) * BL, :t_steps, :]      # [16, t, 128]
        xt = np.ascontiguousarray(xs.transpose(2, 1, 0))  # [128, t, 16]
        m = dict(shared)
        m["xin"] = xt.reshape(P, t_steps * BL).astype(bf16)
        in_maps.append(m)
    return in_maps


def _unpack_output(results):
    out = np.empty((B, H), np.float32)
    for c in range(NCORES):
        o = results[c]["hout"].reshape(P, 2, BL)          # [p, k, b]
        out[c * BL:(c + 1) * BL, :] = o.transpose(2, 1, 0).reshape(BL, H)
    return out


def run(input, W_ih, W_hh, b_ih, b_hh, t_steps: int = T, trace: bool = False):
    nc = _get_program(t_steps)
    in_maps = _pack_inputs(input, W_ih, W_hh, b_ih, b_hh, t_steps)
    res = run_bass_kernel_spmd(
        nc, in_maps, core_ids=list(range(NCORES)), trace=trace
    )
    return _unpack_output(res.results), res


def kernel(input, W_ih, W_hh, b_ih, b_hh):
    out, _ = run(input, W_ih, W_hh, b_ih, b_hh)
    return out


def bench(input, W_ih, W_hh, b_ih, b_hh, reps_hi: int = 41, iters: int = 5):
    """Estimate on-device time: wall(R=reps_hi) - wall(R=1) over cached
    executables, divided by (reps_hi - 1). Returns ns."""
    import time as _time

    in_maps = _pack_inputs(input, W_ih, W_hh, b_ih, b_hh, T)
    nc1 = _get_program(T, 1)
    ncR = _get_program(T, reps_hi)

    def timed(nc):
        best = float("inf")
        for _ in range(iters):
            t0 = _time.perf_counter()
            run_bass_kernel_spmd(nc, in_maps, core_ids=list(range(NCORES)))
            best = min(best, _time.perf_counter() - t0)
        return best

    run_bass_kernel_spmd(nc1, in_maps, core_ids=list(range(NCORES)))
    run_bass_kernel_spmd(ncR, in_maps, core_ids=list(range(NCORES)))
    t1 = timed(nc1)
    tR = timed(ncR)
    ns = (tR - t1) / (reps_hi - 1) * 1e9
    print(f"wall R=1: {t1*1e3:.1f} ms   wall R={reps_hi}: {tR*1e3:.1f} ms")
    return ns
